# revision 40
# baseline (speedup 1.0000x reference)
"""Trainium2 Bass kernel for the CliffordKAN layer problem (schedule rework).

Measured (single-shot NTFF, core 0): 45.9-46.0 us (best family
45.5-46.0 across configs) vs 47.4 us for the prior fp8 baseline
(kernel_v1_backup.py) measured identically on the same day/hardware;
run-to-run spread is ~+-3 us, dominated by the PE HAM clock-gate
phase, and one observed run was ~1.2x slower chip-wide (P0 power-state
downclock). Weight DMA streams gap-free 8.4->35.7 us at 320-358 GB/s;
TensorE is the critical path (ends ~40.5 us; total TE idle < 1.5 us),
then ~0.8 us copy+store and the ~8 us fixed epilogue.

Structure vs the fp8 baseline:

- W k-tiles are ordered i-major (q = i*NGB + gb) so rbf chunks are
  consumed in production order: W chunk c only needs exp windows
  nb <= ~0.9c, letting the ScalarE exp pipeline lead the weight stream
  (gb-major needed HALF of all rbf before W chunk 0 finished).
- All W DMAs are issued upfront on the Sync HWDGE ring (11 chunks,
  small head + small tail); the 8.4 MB weight stream then runs gap-free
  at 320-358 GB/s. Only fat-descriptor transfers ride this ring:
  24-partition or [128, big-contiguous] shapes. (128-partition *sliced*
  DMAs emit 128 tiny strided descriptors and collapse the queue to
  ~60-90 GB/s - that pathology cost two failed iterations.)
- The exponent operands ship as 24-partition tensors and are replicated
  to partition offsets 0/32/64/96 ON-DEVICE via a selection-matrix
  matmul (sel.T @ x -> psum, idle-DVE cast back to SBUF). SBUF->SBUF
  DMA replication is ~80 GB/s (24/128 ports) and stalled the PE.
- rbf exponent matmuls are 4-way row-tiled (tile_position=(32r, 0),
  K=24 per tile): one 512-column pass computes all four g-blocks'
  exponents concurrently (~4x less PE time than the baseline's
  isolated per-block matmuls).
- exp is evaluated in [128, 2x512] activations (2 per window, 16 total)
  to amortize ACT fixed overhead; all 32 rbf chunks stay resident in
  SBUF (no recompute).
- ~4.3 us of dense garbage N=512 matmuls at the start (plus 2x2
  fillers at the early window stalls) lift the PE HAM clock gate
  (1.2 -> 2.4 GHz cold/warm) before the weight stream arrives; without
  them a large fraction of the 512 FD=64 matmuls run at half clock
  (warm cadence is ~29-37 ns per LDWEIGHTS+MATMUL pair, cold ~53-80).

Fixed costs outside kernel control: ~5.9 us NEFF prologue (excluded
from exec_time), ~8 us end-of-NEFF semaphore-reset butterfly (255
individual $S[n]=0 ops in a framework epilogue block - included in
exec_time; only 14 sems are actually used by this kernel).
"""

import numpy as np

from concourse import bacc, bass, mybir  # noqa: F401
from concourse.bass_utils import run_bass_kernel_spmd
from concourse.tile import TileContext

B, I, O, G, X = 64, 64, 64, 4096, 4
NCORES = 8
GS = G // NCORES            # grid points per core = 512
NGB = GS // 128             # g-blocks per core = 4
NKT = NGB * I               # big-matmul k-tiles per core = 256 (i-major)
OX = O * X                  # 256
IB = I * B                  # 4096
NW = IB // 512              # rbf windows (nb) = 8

# W DMA chunk sizes in k-tiles: small head (start matmuls early),
# small tail (short drain after the last chunk lands).
CHUNKS = [16, 28, 28, 28, 28, 28, 28, 28, 28, 8, 8]
assert sum(CHUNKS) == NKT
CHUNK_START = [sum(CHUNKS[:c]) for c in range(len(CHUNKS))]

_nc_cache = None
last_results = None


def _cayley():
    C = np.zeros((4, 4, 4), dtype=np.float32)
    entries = [
        (0, 0, 0, 1), (0, 1, 1, 1), (0, 2, 2, 1), (0, 3, 3, 1),
        (1, 0, 1, 1), (1, 1, 0, 1), (1, 2, 3, 1), (1, 3, 2, 1),
        (2, 0, 2, 1), (2, 1, 3, -1), (2, 2, 0, 1), (2, 3, 1, -1),
        (3, 0, 3, 1), (3, 1, 2, -1), (3, 2, 1, 1), (3, 3, 0, -1),
    ]
    for xx, y, z, s in entries:
        C[xx, y, z] = s
    return C


def _build_bass():
    global _nc_cache
    if _nc_cache is not None:
        return _nc_cache

    nc = bacc.Bacc(
        "TRN2", target_bir_lowering=False, debug=False, num_devices=NCORES
    )
    f32 = mybir.dt.float32
    bf16 = mybir.dt.bfloat16
    f16 = mybir.dt.float16
    f8 = mybir.dt.float8e3

    wt = nc.dram_tensor("wt", [128, NKT * OX], f8, kind="ExternalInput")
    # gs: ga24 (cols 0:GS) + the partition-replication selection matrix
    # sel[k, 32r+k] = 1 (cols GS:GS+128), both on 24 partitions so the DMA
    # uses fat contiguous per-partition descriptors.
    ga = nc.dram_tensor("ga", [24, GS + 128], bf16, kind="ExternalInput")
    # xa ships host-replicated to partition offsets 0/32/64/96 and is
    # fetched as ONE unsliced [128, 8KB-contiguous] DMA (the fast
    # descriptor shape) so no PE replication matmuls are needed.
    xa = nc.dram_tensor("xa", [128, IB], bf16, kind="ExternalInput")
    ls = nc.dram_tensor("ls", [128, 3, B], f16, kind="ExternalInput")
    ws = nc.dram_tensor("ws", [128, 3, OX], f16, kind="ExternalInput")
    out = nc.dram_tensor("out", [128, 2, B], f32, kind="ExternalOutput")

    with TileContext(nc) as tc:
        with (
            tc.tile_pool(name="const", bufs=1) as const,
            tc.tile_pool(name="wpool", bufs=len(CHUNKS)) as wpool,
            tc.tile_pool(name="rpool", bufs=NW) as rpool,
            tc.tile_pool(name="psa", bufs=2, space="PSUM") as psa_pool,
            tc.tile_pool(name="rep", bufs=3, space="PSUM") as rep_pool,
            tc.tile_pool(name="pso", bufs=1, space="PSUM") as pso_pool,
        ):
            # --- PE warm-up: ~3.4us of dense garbage N=512 matmuls flips
            # the HAM clock gate (1.2 -> 2.4 GHz) before the real stream;
            # without it ~25% of the kernel's matmuls run at half clock.
            warm_t = const.tile([128, 512], bf16)
            nc.vector.memset(warm_t[:], 1.0)
            warm_ps = rep_pool.tile([128, 512], f32, tag="rep")

            def warm(n):
                for _ in range(n):
                    nc.tensor.matmul(
                        warm_ps[:], warm_t[:, 0:128], warm_t[:],
                        start=True, stop=True,
                    )

            warm(10)

            # --- DMA issue order. Sync ring: exponent operands first (fat
            # 24-partition descriptors, land while the warm-up burst runs),
            # then the W stream back-to-back.
            gs_t = const.tile([24, GS + 128], bf16)
            nc.sync.dma_start(gs_t[:], ga[:])
            xa4_t = const.tile([128, IB], bf16)
            w_t = []
            for c, ck in enumerate(CHUNKS):
                t = wpool.tile([128, ck, OX], f8)
                nc.sync.dma_start(
                    t[:], wt[:, CHUNK_START[c] * OX:(CHUNK_START[c] + ck) * OX]
                )
                w_t.append(t)
                if c == 0:
                    # slot the 1MB xa fetch right behind the first W chunk:
                    # windows can't start before the warm-up ends anyway.
                    nc.sync.dma_start(xa4_t[:], xa[:])

            # Scalar ring: just the two silu const tensors.
            ls_t = const.tile([128, 3, B], f16)
            nc.scalar.dma_start(ls_t[:], ls[:])
            ws_t = const.tile([128, 3, OX], f16)
            nc.scalar.dma_start(ws_t[:], ws[:])

            pso = pso_pool.tile([128, 2, B], f32)

            # On-device partition replication of ga only: sel.T @ ga copies
            # its 24 rows to partition offsets 0/32/64/96 (psum), DVE
            # writes them back to SBUF.
            sel = gs_t[0:24, GS:GS + 128]
            ga4_t = const.tile([128, GS], bf16)
            ga_ps = rep_pool.tile([128, GS], f32, tag="rep", name="ga_ps")
            nc.tensor.matmul(ga_ps[:], sel, gs_t[0:24, 0:GS],
                             start=True, stop=True)
            nc.vector.tensor_copy(ga4_t[:], ga_ps[:])
            rbf = [
                rpool.tile([128, NGB, 512], f16, name=f"rbf{nb}", tag="rbf")
                for nb in range(NW)
            ]

            def rbf_window(nb, quarters=False):
                ps = [
                    psa_pool.tile([128, 2, 512], f32, name=f"ps{nb}_{j}",
                                  tag="ps")
                    for j in range(2)
                ]
                for r in range(NGB):
                    nc.tensor.matmul(
                        ps[r // 2][:, r % 2, :],
                        ga4_t[32 * r:32 * r + 24, r * 128:(r + 1) * 128],
                        xa4_t[32 * r:32 * r + 24, nb * 512:(nb + 1) * 512],
                        start=True, stop=True,
                        tile_position=(32 * r, 0),
                    )
                if quarters:
                    # per-g-block exps: chunks consume the window g-block by
                    # g-block, so quarter granularity unblocks them ~1.5us
                    # earlier at pipeline start.
                    for j in range(4):
                        nc.scalar.activation(
                            rbf[nb][:, j, :], ps[j // 2][:, j % 2, :],
                            mybir.ActivationFunctionType.Exp,
                        )
                else:
                    for j in range(2):
                        nc.scalar.activation(
                            rbf[nb][:, 2 * j:2 * j + 2, :], ps[j][:],
                            mybir.ActivationFunctionType.Exp,
                        )

            def big_chunk(c, stop=False):
                # gb-major emission within the chunk so early chunks consume
                # quarter-granularity exps as soon as each g-block is ready.
                order = sorted(range(CHUNKS[c]),
                               key=lambda t: (CHUNK_START[c] + t) % NGB)
                for n, t in enumerate(order):
                    q = CHUNK_START[c] + t
                    i, gb = divmod(q, NGB)
                    nb, il = divmod(i, 8)
                    for h in range(2):
                        # single start=True: pending-zero marking covers the
                        # whole 2KB psum region (both h-halves), so only the
                        # first matmul of the accumulation may set it.
                        nc.tensor.matmul(
                            pso[:, h, :],
                            w_t[c][:, t, h * 128:(h + 1) * 128],
                            rbf[nb][:, gb, il * B:(il + 1) * B],
                            start=(c == 0 and n == 0 and h == 0),
                            stop=(stop and n == len(order) - 1 and h == 1),
                            skip_group_check=True,
                        )

            # Interleave: rep runs 3 windows ahead so the DVE copy is done
            # before the window's matmuls; windows stay ahead of the chunks
            # that consume them (chunk c needs windows nb <= (1 + 7c)/8).
            rbf_window(0, quarters=True)
            warm(2)
            rbf_window(1, quarters=True)
            warm(2)
            big_chunk(0)

            # silu branch: 6 small matmuls folded into the accumulation
            # early (ls/ws land by ~10us) so they don't extend the tail.
            for s in range(3):
                for h in range(2):
                    nc.tensor.matmul(
                        pso[:, h, :],
                        ws_t[:, s, h * 128:(h + 1) * 128],
                        ls_t[:, s, :],
                        start=False,
                        stop=False,
                        skip_group_check=True,
                    )

            for c in range(1, len(CHUNKS)):
                if c + 1 < NW:
                    rbf_window(c + 1)
                if c <= 5:
                    # no-dep filler keeps PE duty high through the exp-gated
                    # early chunks so the HAM clock gate stays released.
                    warm(1)
                big_chunk(c, stop=(c == len(CHUNKS) - 1))

            out_t = const.tile([128, 2, B], f32)
            nc.vector.tensor_copy(out_t[:], pso[:])
            nc.scalar.dma_start(out[:], out_t[:])

    nc.compile()
    _nc_cache = nc
    return nc


def make_core_inputs(x, grid, weights, silu_weight, silu_bias):
    """Host-side shard + layout prep. Returns list of 8 input dicts."""
    x = np.ascontiguousarray(x, dtype=np.float32)
    grid = np.ascontiguousarray(grid, dtype=np.float32)
    weights = np.ascontiguousarray(weights, dtype=np.float32)
    silu_weight = np.ascontiguousarray(silu_weight, dtype=np.float32)
    silu_bias = np.ascontiguousarray(silu_bias, dtype=np.float32)

    import ml_dtypes

    def split24(a6, pattern):
        hi = a6.astype(ml_dtypes.bfloat16)
        lo = (a6 - hi.astype(np.float32)).astype(ml_dtypes.bfloat16)
        parts = {"h": hi, "l": lo}
        return np.ascontiguousarray(
            np.concatenate([parts[p] for p in pattern], axis=0)
        )

    # xa: (24, I*B), column j = i*B + b; rows = bf16 split "hhll"
    xt = x.transpose(1, 0, 2)                       # (I, B, X)
    xa6 = np.empty((6, IB), dtype=np.float32)
    xa6[0:4] = xt.reshape(IB, X).T
    xa6[4] = 1.0
    xa6[5] = -(xt ** 2).sum(-1).reshape(IB)
    xa24 = split24(xa6, "hhll")                     # (24, IB) bf16
    xa4 = np.zeros((128, IB), dtype=xa24.dtype)     # replicated at 32r
    for r in range(4):
        xa4[32 * r:32 * r + 24] = xa24

    # partition-replication selection matrix (used on-device for ga):
    # sel.T @ v puts v's 24 rows at partition offsets 0/32/64/96
    sel = np.zeros((24, 128), dtype=np.float32)
    for r in range(4):
        for k in range(24):
            sel[k, 32 * r + k] = 1.0

    # silu lhsT (core 0 only): rows k2 = i*4+y -> silu(x)[b,i,y]; row 256 -> 1
    sx = x / (1.0 + np.exp(-x))                     # silu(x), (B, I, X)
    lsf = np.zeros((384, B), dtype=np.float32)
    lsf[0:256] = sx.transpose(1, 2, 0).reshape(256, B)
    lsf[256] = 1.0
    ls0 = np.ascontiguousarray(
        lsf.reshape(3, 128, B).transpose(1, 0, 2)).astype(np.float16)
    lsz = np.zeros_like(ls0)

    # silu rhs: M2[(i,y),(o,z)] = sum_x silu_weight[i,o,x]*C[x,y,z]; row 256 bias
    C = _cayley()
    m2 = np.einsum("iox,xyz->iyoz", silu_weight, C).reshape(256, OX)
    wsf = np.zeros((384, OX), dtype=np.float32)
    wsf[0:256] = m2
    wsf[256] = silu_bias.sum(axis=0).reshape(OX)
    ws = np.ascontiguousarray(
        wsf.reshape(3, 128, OX).transpose(1, 0, 2)).astype(np.float16)

    in_maps = []
    for c in range(NCORES):
        gsl = slice(c * GS, (c + 1) * GS)
        gc = grid[gsl]                              # (GS, 4)
        ga6 = np.empty((6, GS), dtype=np.float32)
        ga6[0:4] = 2.0 * gc.T
        ga6[4] = -(gc ** 2).sum(-1)
        ga6[5] = 1.0
        ga24 = split24(ga6, "hlhl")                 # (24, GS) bf16
        gs = np.zeros((24, GS + 128), dtype=ml_dtypes.bfloat16)
        gs[:, 0:GS] = ga24
        gs[:, GS:] = sel.astype(ml_dtypes.bfloat16)

        # W slab -> flat [128, NKT*OX], k-tile q = i*NGB + gb (i-major)
        warr = weights[:, :, gsl, :].transpose(0, 2, 1, 3).reshape(I, GS, OX)
        tmp = warr.reshape(I * NGB, 128, OX).transpose(1, 0, 2)
        wtc = np.ascontiguousarray(
            tmp.reshape(128, NKT * OX)).astype(ml_dtypes.float8_e3m4)

        in_maps.append({
            "wt": wtc,
            "ga": gs,
            "xa": xa4,
            "ls": ls0 if c == 0 else lsz,
            "ws": ws,
        })
    return in_maps


def kernel(x, grid, weights, silu_weight, silu_bias):
    global last_results
    nc = _build_bass()
    in_maps = make_core_inputs(x, grid, weights, silu_weight, silu_bias)
    res = run_bass_kernel_spmd(nc, in_maps, list(range(NCORES)))
    last_results = res
    acc = np.zeros((B, OX), dtype=np.float32)
    for r in res.results:
        o = r["out"]                                # (128, 2, B)
        acc += o.transpose(2, 1, 0).reshape(B, OX)
    return acc.reshape(B, O, X)


# revision 42
# speedup vs baseline: 1.1009x; 1.1009x over previous
"""Trainium2 Bass kernel for the CliffordKAN layer problem (schedule rework).

Measured (single-shot NTFF, core 0): 45.9-46.0 us (best family
45.5-46.0 across configs) vs 47.4 us for the prior fp8 baseline
(kernel_v1_backup.py) measured identically on the same day/hardware;
run-to-run spread is ~+-3 us, dominated by the PE HAM clock-gate
phase, and one observed run was ~1.2x slower chip-wide (P0 power-state
downclock). Weight DMA streams gap-free 8.4->35.7 us at 320-358 GB/s;
TensorE is the critical path (ends ~40.5 us; total TE idle < 1.5 us),
then ~0.8 us copy+store and the ~8 us fixed epilogue.

Structure vs the fp8 baseline:

- W k-tiles are ordered i-major (q = i*NGB + gb) so rbf chunks are
  consumed in production order: W chunk c only needs exp windows
  nb <= ~0.9c, letting the ScalarE exp pipeline lead the weight stream
  (gb-major needed HALF of all rbf before W chunk 0 finished).
- All W DMAs are issued upfront on the Sync HWDGE ring (11 chunks,
  small head + small tail); the 8.4 MB weight stream then runs gap-free
  at 320-358 GB/s. Only fat-descriptor transfers ride this ring:
  24-partition or [128, big-contiguous] shapes. (128-partition *sliced*
  DMAs emit 128 tiny strided descriptors and collapse the queue to
  ~60-90 GB/s - that pathology cost two failed iterations.)
- The exponent operands ship as 24-partition tensors and are replicated
  to partition offsets 0/32/64/96 ON-DEVICE via a selection-matrix
  matmul (sel.T @ x -> psum, idle-DVE cast back to SBUF). SBUF->SBUF
  DMA replication is ~80 GB/s (24/128 ports) and stalled the PE.
- rbf exponent matmuls are 4-way row-tiled (tile_position=(32r, 0),
  K=24 per tile): one 512-column pass computes all four g-blocks'
  exponents concurrently (~4x less PE time than the baseline's
  isolated per-block matmuls).
- exp is evaluated in [128, 2x512] activations (2 per window, 16 total)
  to amortize ACT fixed overhead; all 32 rbf chunks stay resident in
  SBUF (no recompute).
- ~4.3 us of dense garbage N=512 matmuls at the start (plus 2x2
  fillers at the early window stalls) lift the PE HAM clock gate
  (1.2 -> 2.4 GHz cold/warm) before the weight stream arrives; without
  them a large fraction of the 512 FD=64 matmuls run at half clock
  (warm cadence is ~29-37 ns per LDWEIGHTS+MATMUL pair, cold ~53-80).

Fixed costs outside kernel control: ~5.9 us NEFF prologue (excluded
from exec_time), ~8 us end-of-NEFF semaphore-reset butterfly (255
individual $S[n]=0 ops in a framework epilogue block - included in
exec_time; only 14 sems are actually used by this kernel).
"""

import numpy as np

from concourse import bacc, bass, mybir  # noqa: F401
from concourse.bass_utils import run_bass_kernel_spmd
from concourse.tile import TileContext

B, I, O, G, X = 64, 64, 64, 4096, 4
NCORES = 8
GS = G // NCORES            # grid points per core = 512
NGB = GS // 128             # g-blocks per core = 4
NKT = NGB * I               # big-matmul k-tiles per core = 256 (i-major)
OX = O * X                  # 256
IB = I * B                  # 4096
NW = IB // 512              # rbf windows (nb) = 8

# W DMA chunk sizes in k-tiles: small head (start matmuls early),
# small tail (short drain after the last chunk lands).
CHUNKS = [16, 28, 28, 28, 28, 28, 28, 28, 28, 8, 8]
assert sum(CHUNKS) == NKT
CHUNK_START = [sum(CHUNKS[:c]) for c in range(len(CHUNKS))]

_nc_cache = None
last_results = None


def _cayley():
    C = np.zeros((4, 4, 4), dtype=np.float32)
    entries = [
        (0, 0, 0, 1), (0, 1, 1, 1), (0, 2, 2, 1), (0, 3, 3, 1),
        (1, 0, 1, 1), (1, 1, 0, 1), (1, 2, 3, 1), (1, 3, 2, 1),
        (2, 0, 2, 1), (2, 1, 3, -1), (2, 2, 0, 1), (2, 3, 1, -1),
        (3, 0, 3, 1), (3, 1, 2, -1), (3, 2, 1, 1), (3, 3, 0, -1),
    ]
    for xx, y, z, s in entries:
        C[xx, y, z] = s
    return C


def _build_bass():
    global _nc_cache
    if _nc_cache is not None:
        return _nc_cache

    nc = bacc.Bacc(
        "TRN2", target_bir_lowering=False, debug=False, num_devices=NCORES
    )
    f32 = mybir.dt.float32
    bf16 = mybir.dt.bfloat16
    f16 = mybir.dt.float16
    f8 = mybir.dt.float8e3

    wt = nc.dram_tensor("wt", [128, NKT * OX], f8, kind="ExternalInput")
    # gs: ga24 (cols 0:GS) + the partition-replication selection matrix
    # sel[k, 32r+k] = 1 (cols GS:GS+128), both on 24 partitions so the DMA
    # uses fat contiguous per-partition descriptors.
    ga = nc.dram_tensor("ga", [24, GS + 128], bf16, kind="ExternalInput")
    xa = nc.dram_tensor("xa", [24, IB], bf16, kind="ExternalInput")
    ls = nc.dram_tensor("ls", [128, 3, B], f16, kind="ExternalInput")
    ws = nc.dram_tensor("ws", [128, 3, OX], f16, kind="ExternalInput")
    out = nc.dram_tensor("out", [128, 2, B], f32, kind="ExternalOutput")

    with TileContext(nc) as tc:
        with (
            tc.tile_pool(name="const", bufs=1) as const,
            tc.tile_pool(name="wpool", bufs=len(CHUNKS)) as wpool,
            tc.tile_pool(name="rpool", bufs=NW) as rpool,
            tc.tile_pool(name="psa", bufs=2, space="PSUM") as psa_pool,
            tc.tile_pool(name="rep", bufs=3, space="PSUM") as rep_pool,
            tc.tile_pool(name="pso", bufs=1, space="PSUM") as pso_pool,
        ):
            # --- PE warm-up: ~3.4us of dense garbage N=512 matmuls flips
            # the HAM clock gate (1.2 -> 2.4 GHz) before the real stream;
            # without it ~25% of the kernel's matmuls run at half clock.
            warm_t = const.tile([128, 512], bf16)
            nc.vector.memset(warm_t[:], 1.0)
            warm_ps = rep_pool.tile([128, 512], f32, tag="rep")

            def warm(n):
                for _ in range(n):
                    nc.tensor.matmul(
                        warm_ps[:], warm_t[:, 0:128], warm_t[:],
                        start=True, stop=True,
                    )

            warm(7)

            # --- DMA issue order. Sync ring: exponent operands first (fat
            # 24-partition descriptors, land while the warm-up burst runs),
            # then the W stream back-to-back.
            gs_t = const.tile([24, GS + 128], bf16)
            nc.sync.dma_start(gs_t[:], ga[:])
            xa_t = const.tile([24, IB], bf16)
            nc.sync.dma_start(xa_t[:], xa[:])
            w_t = []
            for c, ck in enumerate(CHUNKS):
                t = wpool.tile([128, ck, OX], f8)
                nc.sync.dma_start(
                    t[:], wt[:, CHUNK_START[c] * OX:(CHUNK_START[c] + ck) * OX]
                )
                w_t.append(t)

            # Scalar ring: just the two silu const tensors.
            ls_t = const.tile([128, 3, B], f16)
            nc.scalar.dma_start(ls_t[:], ls[:])
            ws_t = const.tile([128, 3, OX], f16)
            nc.scalar.dma_start(ws_t[:], ws[:])

            pso = pso_pool.tile([128, 2, B], f32)

            # On-device partition replication: sel.T @ x copies 24 rows to
            # partition offsets 0/32/64/96 (psum), DVE writes them back to
            # SBUF. These early matmuls double as the PE HAM warm-up.
            sel = gs_t[0:24, GS:GS + 128]
            ga4_t = const.tile([128, GS], bf16)
            ga_ps = rep_pool.tile([128, GS], f32, tag="rep", name="ga_ps")
            nc.tensor.matmul(ga_ps[:], sel, gs_t[0:24, 0:GS],
                             start=True, stop=True)
            nc.vector.tensor_copy(ga4_t[:], ga_ps[:])
            xa4_t = const.tile([128, IB], bf16)

            def rep_window(nb):
                ps = rep_pool.tile([128, 512], f32, tag="rep",
                                   name=f"rep{nb}")
                nc.tensor.matmul(
                    ps[:], sel, xa_t[:, nb * 512:(nb + 1) * 512],
                    start=True, stop=True,
                )
                nc.vector.tensor_copy(xa4_t[:, nb * 512:(nb + 1) * 512],
                                      ps[:])
            rbf = [
                rpool.tile([128, NGB, 512], f16, name=f"rbf{nb}", tag="rbf")
                for nb in range(NW)
            ]

            def rbf_window(nb, quarters=False):
                ps = [
                    psa_pool.tile([128, 2, 512], f32, name=f"ps{nb}_{j}",
                                  tag="ps")
                    for j in range(2)
                ]
                for r in range(NGB):
                    nc.tensor.matmul(
                        ps[r // 2][:, r % 2, :],
                        ga4_t[32 * r:32 * r + 24, r * 128:(r + 1) * 128],
                        xa4_t[32 * r:32 * r + 24, nb * 512:(nb + 1) * 512],
                        start=True, stop=True,
                        tile_position=(32 * r, 0),
                    )
                if quarters:
                    # per-g-block exps: chunks consume the window g-block by
                    # g-block, so quarter granularity unblocks them ~1.5us
                    # earlier at pipeline start.
                    for j in range(4):
                        nc.scalar.activation(
                            rbf[nb][:, j, :], ps[j // 2][:, j % 2, :],
                            mybir.ActivationFunctionType.Exp,
                        )
                else:
                    for j in range(2):
                        nc.scalar.activation(
                            rbf[nb][:, 2 * j:2 * j + 2, :], ps[j][:],
                            mybir.ActivationFunctionType.Exp,
                        )

            def big_chunk(c, stop=False):
                # gb-major emission within the chunk so early chunks consume
                # quarter-granularity exps as soon as each g-block is ready.
                order = sorted(range(CHUNKS[c]),
                               key=lambda t: (CHUNK_START[c] + t) % NGB)
                for n, t in enumerate(order):
                    q = CHUNK_START[c] + t
                    i, gb = divmod(q, NGB)
                    nb, il = divmod(i, 8)
                    for h in range(2):
                        # single start=True: pending-zero marking covers the
                        # whole 2KB psum region (both h-halves), so only the
                        # first matmul of the accumulation may set it.
                        nc.tensor.matmul(
                            pso[:, h, :],
                            w_t[c][:, t, h * 128:(h + 1) * 128],
                            rbf[nb][:, gb, il * B:(il + 1) * B],
                            start=(c == 0 and n == 0 and h == 0),
                            stop=(stop and n == len(order) - 1 and h == 1),
                            skip_group_check=True,
                        )

            # Interleave: rep runs 3 windows ahead so the DVE copy is done
            # before the window's matmuls; windows stay ahead of the chunks
            # that consume them (chunk c needs windows nb <= (1 + 7c)/8).
            rep_window(0)
            rep_window(1)
            rep_window(2)
            rbf_window(0, quarters=True)
            warm(2)
            rep_window(3)
            rbf_window(1, quarters=True)
            warm(2)
            big_chunk(0)

            # silu branch: 6 small matmuls folded into the accumulation
            # early (ls/ws land by ~10us) so they don't extend the tail.
            for s in range(3):
                for h in range(2):
                    nc.tensor.matmul(
                        pso[:, h, :],
                        ws_t[:, s, h * 128:(h + 1) * 128],
                        ls_t[:, s, :],
                        start=False,
                        stop=False,
                        skip_group_check=True,
                    )

            for c in range(1, len(CHUNKS)):
                if c + 3 < NW:
                    rep_window(c + 3)
                if c + 1 < NW:
                    rbf_window(c + 1)
                if c <= 5:
                    # no-dep filler keeps PE duty high through the exp-gated
                    # early chunks so the HAM clock gate stays released.
                    warm(1)
                big_chunk(c, stop=(c == len(CHUNKS) - 1))

            out_t = const.tile([128, 2, B], f32)
            nc.vector.tensor_copy(out_t[:], pso[:])
            nc.scalar.dma_start(out[:], out_t[:])

    nc.compile()
    _nc_cache = nc
    return nc


def make_core_inputs(x, grid, weights, silu_weight, silu_bias):
    """Host-side shard + layout prep. Returns list of 8 input dicts."""
    x = np.ascontiguousarray(x, dtype=np.float32)
    grid = np.ascontiguousarray(grid, dtype=np.float32)
    weights = np.ascontiguousarray(weights, dtype=np.float32)
    silu_weight = np.ascontiguousarray(silu_weight, dtype=np.float32)
    silu_bias = np.ascontiguousarray(silu_bias, dtype=np.float32)

    import ml_dtypes

    def split24(a6, pattern):
        hi = a6.astype(ml_dtypes.bfloat16)
        lo = (a6 - hi.astype(np.float32)).astype(ml_dtypes.bfloat16)
        parts = {"h": hi, "l": lo}
        return np.ascontiguousarray(
            np.concatenate([parts[p] for p in pattern], axis=0)
        )

    # xa: (24, I*B), column j = i*B + b; rows = bf16 split "hhll"
    xt = x.transpose(1, 0, 2)                       # (I, B, X)
    xa6 = np.empty((6, IB), dtype=np.float32)
    xa6[0:4] = xt.reshape(IB, X).T
    xa6[4] = 1.0
    xa6[5] = -(xt ** 2).sum(-1).reshape(IB)
    xa24 = split24(xa6, "hhll")                     # (24, IB) bf16

    # partition-replication selection matrix: sel.T @ v puts v's 24 rows
    # at partition offsets 0/32/64/96
    sel = np.zeros((24, 128), dtype=np.float32)
    for r in range(4):
        for k in range(24):
            sel[k, 32 * r + k] = 1.0

    # silu lhsT (core 0 only): rows k2 = i*4+y -> silu(x)[b,i,y]; row 256 -> 1
    sx = x / (1.0 + np.exp(-x))                     # silu(x), (B, I, X)
    lsf = np.zeros((384, B), dtype=np.float32)
    lsf[0:256] = sx.transpose(1, 2, 0).reshape(256, B)
    lsf[256] = 1.0
    ls0 = np.ascontiguousarray(
        lsf.reshape(3, 128, B).transpose(1, 0, 2)).astype(np.float16)
    lsz = np.zeros_like(ls0)

    # silu rhs: M2[(i,y),(o,z)] = sum_x silu_weight[i,o,x]*C[x,y,z]; row 256 bias
    C = _cayley()
    m2 = np.einsum("iox,xyz->iyoz", silu_weight, C).reshape(256, OX)
    wsf = np.zeros((384, OX), dtype=np.float32)
    wsf[0:256] = m2
    wsf[256] = silu_bias.sum(axis=0).reshape(OX)
    ws = np.ascontiguousarray(
        wsf.reshape(3, 128, OX).transpose(1, 0, 2)).astype(np.float16)

    in_maps = []
    for c in range(NCORES):
        gsl = slice(c * GS, (c + 1) * GS)
        gc = grid[gsl]                              # (GS, 4)
        ga6 = np.empty((6, GS), dtype=np.float32)
        ga6[0:4] = 2.0 * gc.T
        ga6[4] = -(gc ** 2).sum(-1)
        ga6[5] = 1.0
        ga24 = split24(ga6, "hlhl")                 # (24, GS) bf16
        gs = np.zeros((24, GS + 128), dtype=ml_dtypes.bfloat16)
        gs[:, 0:GS] = ga24
        gs[:, GS:] = sel.astype(ml_dtypes.bfloat16)

        # W slab -> flat [128, NKT*OX], k-tile q = i*NGB + gb (i-major)
        warr = weights[:, :, gsl, :].transpose(0, 2, 1, 3).reshape(I, GS, OX)
        tmp = warr.reshape(I * NGB, 128, OX).transpose(1, 0, 2)
        wtc = np.ascontiguousarray(
            tmp.reshape(128, NKT * OX)).astype(ml_dtypes.float8_e3m4)

        in_maps.append({
            "wt": wtc,
            "ga": gs,
            "xa": xa24,
            "ls": ls0 if c == 0 else lsz,
            "ws": ws,
        })
    return in_maps


def kernel(x, grid, weights, silu_weight, silu_bias):
    global last_results
    nc = _build_bass()
    in_maps = make_core_inputs(x, grid, weights, silu_weight, silu_bias)
    res = run_bass_kernel_spmd(nc, in_maps, list(range(NCORES)))
    last_results = res
    acc = np.zeros((B, OX), dtype=np.float32)
    for r in res.results:
        o = r["out"]                                # (128, 2, B)
        acc += o.transpose(2, 1, 0).reshape(B, OX)
    return acc.reshape(B, O, X)
